# revision 45
# baseline (speedup 1.0000x reference)
"""Trainium2 Bass kernel for nn_BAE (VAE-style encoder/decoder with fused
scatter-add attention heads), data-parallel over 8 NeuronCores.

Key algebraic simplifications applied on host:
- seq_len==1 attention: softmax over a singleton axis is exactly 1, so the
  attention output equals the v-projection; q/k matmuls are dropped.
- Eval-mode BatchNorm is an affine transform; it is folded into the next
  layer's weights/biases, so no BN ops run on device.
- The edge symmetrization (out + out^T)/2 is linear; it is folded into
  dec2's weight/bias.
- log.mean(axis=1) is folded into the log-attention v-weights (tiled /32).
- The fixed-index scatter-adds become one 0/1 scatter matrix S applied as
  extra PSUM-accumulating matmuls in the dec2 output group.

Device pipeline (per core, feature-major, bf16 matmul / fp32 PSUM):
  xT -> relu(enc) -> hT -> mu|lv -> zT -> relu(dec1) -> h2T -> dec2 + S@a
  aT = [optical@Wopt | logflat@Wlm] computed the same way.
"""

import os
import numpy as np

B = 32768
N_CORES = 8
BC = B // N_CORES          # 4096 samples per core
NT = 512                   # samples per block (matmul moving dim)
NBLK = BC // NT            # 8 blocks per core
D_IN = 1024
HID = 1024
LAT = 256
OUT_DIM = 1050
NODE = 30
NF = 5
BN_EPS = 1e-5

_opt_nodes = list(range(20, 30))
_OPT_EDGES = [(i, j) for i in _opt_nodes for j in _opt_nodes if i <= j]   # 55
_log_nodes = [20, 21, 22, 23, 24, 25, 26, 27, 28, 29, 14, 15]
_LOG_EDGES = [(i, j) for i in _log_nodes for j in _log_nodes if i <= j]   # 78

NA0 = 10 + len(_OPT_EDGES)     # 65  (optical head: 10 node + 55 edge)
NA1 = 12 + len(_LOG_EDGES)     # 90  (log head: 12 node + 78 edge)

LAST_RESULTS = None  # test harness reads exec_time_ns from here


def _install_ntff_shim():
    """Provide antenv.axon_hooks if the image lacks it, wiring the NTFF
    profile hook to the axon .so via the boot helper. Makes trace=True
    (BASS_TRACE=1) work instead of crashing on a missing import."""
    import sys
    import types

    try:
        from antenv.axon_hooks import get_axon_ntff_profile_hook  # noqa: F401

        return
    except ImportError:
        pass
    hook = None
    try:
        from trn_agent_boot.trn_boot import _ntff_profile_via_ctypes

        hook = _ntff_profile_via_ctypes("/opt/axon/libaxon_pjrt.so")
    except Exception:
        hook = None
    mod = types.ModuleType("antenv.axon_hooks")
    mod._hook = hook
    mod.get_axon_ntff_profile_hook = lambda: mod._hook
    def _set(h):
        mod._hook = h
    mod.set_axon_ntff_profile_hook = _set
    sys.modules["antenv.axon_hooks"] = mod
    try:
        import antenv

        antenv.axon_hooks = mod
    except ImportError:
        pass


def _make_tile_context_cls():
    """TileContext whose kernel-tail drain splits sem waits one-per-drain
    (this toolchain's walrus rejects >few waits on a CTRL instruction)."""
    import bass_rust
    from concourse.tile import TileContext
    from concourse.vector_clock import ScopedClock

    class TileContextSplitDrain(TileContext):
        def _drain_and_barrier(self, tick_clock, wait_clock):
            drain_inst = self.nc.sync.drain()
            wait_clock.add_sem_waits(
                drain_inst.ins, ScopedClock({None: tick_clock.global_clock})
            )
            si = drain_inst.ins.sync_info
            if si is not None:
                waits = list(si.on_wait or [])
                upds = list(si.on_update or [])
                if len(waits) > 1:
                    drain_inst.ins.sync_info = bass_rust.SyncInfo(
                        on_wait=waits[:1], on_update=upds
                    )
                    for i in range(1, len(waits)):
                        extra = self.nc.sync.drain()
                        extra.ins.sync_info = bass_rust.SyncInfo(
                            on_wait=waits[i : i + 1], on_update=[]
                        )
            self.nc.all_engine_barrier()
            assert self.sems is not None
            popped = self.nc._tile_sem_poison_stack.pop()
            assert popped is self._sem_poison
            self.nc.clear_and_free_semaphores(list(self.sems.allocated().values()))
            self.nc.all_engine_barrier()

    return TileContextSplitDrain


def _prep_host(x, optical, log, p):
    """Fold BN/symmetrization/mean, build scatter matrix, lay out weights
    and inputs in device-ready (feature-major, chunked) form."""
    import ml_dtypes

    bf16 = ml_dtypes.bfloat16
    f32 = np.float32

    def npa(a):
        return np.asarray(a, dtype=f32)

    x = npa(x)
    optical = npa(optical)
    log = npa(log)
    p = {k: npa(v) for k, v in p.items()}

    # ---- BN folds -------------------------------------------------------
    g1 = p["enc_bn_g"] / np.sqrt(p["enc_bn_v"] + BN_EPS)
    b1 = p["enc_bn_b"] - p["enc_bn_m"] * g1
    w_mu = g1[:, None] * p["mu_w"]
    b_mu = p["mu_b"] + b1 @ p["mu_w"]
    w_lv = g1[:, None] * p["lv_w"]
    b_lv = p["lv_b"] + b1 @ p["lv_w"]
    w_mulv = np.concatenate([w_mu, w_lv], axis=1)          # [1024, 512]
    b_mulv = np.concatenate([b_mu, b_lv])                   # [512]

    g2 = p["dec_bn_g"] / np.sqrt(p["dec_bn_v"] + BN_EPS)
    b2 = p["dec_bn_b"] - p["dec_bn_m"] * g2
    W = p["dec2_w"]                                         # [1024, 1050]
    E = W[:, : NODE * NODE].reshape(HID, NODE, NODE)
    Es = (E + E.transpose(0, 2, 1)) * 0.5
    w_sym = np.concatenate([Es.reshape(HID, NODE * NODE), W[:, NODE * NODE :]], axis=1)
    bE = p["dec2_b"][: NODE * NODE].reshape(NODE, NODE)
    bEs = (bE + bE.T) * 0.5
    b_sym = np.concatenate([bEs.reshape(-1), p["dec2_b"][NODE * NODE :]])
    w_dec2 = g2[:, None] * w_sym                            # [1024, 1050]
    b_dec2 = b_sym + b2 @ w_sym                             # [1050]

    # ---- attention v-heads (softmax==1 so out == v) ---------------------
    w_opt = np.concatenate([p["on_vw"], p["oe_vw"]], axis=1)   # [1024, 65]
    b_opt = np.concatenate([p["on_vb"], p["oe_vb"]])           # [65]
    w_cat = np.concatenate([p["ln_vw"], p["le_vw"]], axis=1)   # [9, 90]
    b_lm = np.concatenate([p["ln_vb"], p["le_vb"]])            # [90]
    # fold mean over T_LOG=32: logflat[b] @ w_lm_exp == mean(log) @ w_cat
    w_lm_exp = np.zeros((384, NA1), f32)
    w_lm_exp[:288] = np.tile(w_cat / 32.0, (32, 1))

    # ---- scatter matrices ----------------------------------------------
    S0 = np.zeros((NA0, OUT_DIM), f32)
    for k, n in enumerate(_opt_nodes):
        for f in range(NF):
            S0[k, NODE * NODE + n * NF + f] = 1.0
    for k, (i, j) in enumerate(_OPT_EDGES):
        S0[10 + k, i * NODE + j] = 1.0
    S1 = np.zeros((NA1, OUT_DIM), f32)
    for k, n in enumerate(_log_nodes):
        for f in range(NF):
            S1[k, NODE * NODE + n * NF + f] = 1.0
    for k, (i, j) in enumerate(_LOG_EDGES):
        S1[12 + k, i * NODE + j] = 1.0

    # ---- device layouts -------------------------------------------------
    fp8 = ml_dtypes.float8_e4m3

    def chunk_w(w, kchunks, m, dtype=bf16):
        # [K, M] -> [kchunks, 128, M]
        return np.ascontiguousarray(w.reshape(kchunks, 128, m).astype(dtype))

    # dec2 padded to M=1152 so the DoubleRow pair-dim step is 16B-aligned
    w_dec2_pad = np.zeros((HID, 1152), f32)
    w_dec2_pad[:, :OUT_DIM] = w_dec2

    weights = {
        "wenc": chunk_w(p["enc_w"], 8, HID),
        "wmulv": chunk_w(w_mulv, 8, 2 * LAT),
        "wdec1": chunk_w(p["dec1_w"], 2, HID, fp8),
        "wdec2": chunk_w(w_dec2_pad, 8, 1152, fp8),
        "wopt": chunk_w(w_opt, 8, NA0),
        "wlm": chunk_w(w_lm_exp, 3, NA1),
        "s0": np.ascontiguousarray(S0.astype(bf16)),
        "s1": np.ascontiguousarray(S1.astype(bf16)),
        "benc": np.ascontiguousarray(p["enc_b"].reshape(8, 128).T),
        "bmulv": np.ascontiguousarray(b_mulv.reshape(4, 128).T),
        "bdec1": np.ascontiguousarray(p["dec1_b"].reshape(8, 128).T),
    }
    bdec2_pad = np.zeros(1152, f32)
    bdec2_pad[:OUT_DIM] = b_dec2
    weights["bdec2"] = np.ascontiguousarray(bdec2_pad.reshape(9, 128).T)
    ba = np.zeros((128, 2), f32)
    ba[:NA0, 0] = b_opt
    ba[:NA1, 1] = b_lm
    weights["ba"] = ba

    # inputs: feature-major chunked [kchunks, 128, B], bf16
    xt = np.ascontiguousarray(x.reshape(B, 8, 128).transpose(1, 2, 0).astype(bf16))
    ot = np.ascontiguousarray(optical.reshape(B, 8, 128).transpose(1, 2, 0).astype(bf16))
    lf = np.zeros((B, 384), f32)
    lf[:, :288] = log.reshape(B, 288)
    lt = np.ascontiguousarray(lf.reshape(B, 3, 128).transpose(1, 2, 0).astype(bf16))
    return weights, xt, ot, lt


def _legalize_waits(nc, max_waits=1):
    """This toolchain's walrus rejects instructions carrying more than a
    couple of sync waits. Hoist excess waits onto preceding same-engine
    NoOps (engines execute in order, so semantics are unchanged)."""
    import bass_rust
    import concourse.mybir as mybir

    ctr = 0
    for f in nc.m.functions:
        for bb in f.blocks:
            changed = False
            out = []
            for inst in bb.instructions:
                si = inst.sync_info
                waits = list(si.on_wait or []) if si is not None else []
                if len(waits) > max_waits:
                    changed = True
                    extra = waits[max_waits:]
                    for j in range(0, len(extra), max_waits):
                        ctr += 1
                        nop = mybir.InstNoOp(name=f"waitsplit-{ctr}", ins=[], outs=[])
                        nop.engine = inst.engine
                        nop.sync_info = bass_rust.SyncInfo(
                            on_wait=extra[j : j + max_waits], on_update=[]
                        )
                        out.append(nop)
                    inst.sync_info = bass_rust.SyncInfo(
                        on_wait=waits[:max_waits],
                        on_update=list(si.on_update or []),
                    )
                out.append(inst)
            if changed:
                bb.instructions = out


def _build_graph():
    import concourse.bass as bass
    import concourse.mybir as mybir

    TileCtx = _make_tile_context_cls()
    dt = mybir.dt
    AF = mybir.ActivationFunctionType

    nc = bass.Bass()
    # inputs (per-core shard shapes)
    xt_h = nc.declare_dram_parameter("xt", [8, 128, BC], dt.bfloat16, isOutput=False)
    ot_h = nc.declare_dram_parameter("ot", [8, 128, BC], dt.bfloat16, isOutput=False)
    lt_h = nc.declare_dram_parameter("lt", [3, 128, BC], dt.bfloat16, isOutput=False)
    wenc_h = nc.declare_dram_parameter("wenc", [8, 128, HID], dt.bfloat16, isOutput=False)
    wmulv_h = nc.declare_dram_parameter("wmulv", [8, 128, 2 * LAT], dt.bfloat16, isOutput=False)
    wdec1_h = nc.declare_dram_parameter("wdec1", [2, 128, HID], dt.float8e4, isOutput=False)
    wdec2_h = nc.declare_dram_parameter("wdec2", [8, 128, 1152], dt.float8e4, isOutput=False)
    wopt_h = nc.declare_dram_parameter("wopt", [8, 128, NA0], dt.bfloat16, isOutput=False)
    wlm_h = nc.declare_dram_parameter("wlm", [3, 128, NA1], dt.bfloat16, isOutput=False)
    s0_h = nc.declare_dram_parameter("s0", [NA0, OUT_DIM], dt.bfloat16, isOutput=False)
    s1_h = nc.declare_dram_parameter("s1", [NA1, OUT_DIM], dt.bfloat16, isOutput=False)
    benc_h = nc.declare_dram_parameter("benc", [128, 8], dt.float32, isOutput=False)
    bmulv_h = nc.declare_dram_parameter("bmulv", [128, 4], dt.float32, isOutput=False)
    bdec1_h = nc.declare_dram_parameter("bdec1", [128, 8], dt.float32, isOutput=False)
    bdec2_h = nc.declare_dram_parameter("bdec2", [128, 9], dt.float32, isOutput=False)
    ba_h = nc.declare_dram_parameter("ba", [128, 2], dt.float32, isOutput=False)
    # outputs
    outt_h = nc.declare_dram_parameter("outt", [9, 128, BC], dt.float32, isOutput=True)
    mulvt_h = nc.declare_dram_parameter("mulvt", [4, 128, BC], dt.float32, isOutput=True)

    with TileCtx(nc) as tc:
        with (
            tc.tile_pool(name="const", bufs=1) as cpool,
            tc.tile_pool(name="stream", bufs=2) as spool,
            tc.tile_pool(name="outp", bufs=3) as opool,
            tc.tile_pool(name="psum", bufs=8, space="PSUM") as ppool,
        ):
            xt_r = xt_h[:].rearrange("k p b -> p k b")
            ot_r = ot_h[:].rearrange("k p b -> p k b")
            lt_r = lt_h[:].rearrange("k p b -> p k b")
            outt_r = outt_h[:].rearrange("c p b -> p c b")
            mulvt_r = mulvt_h[:].rearrange("c p b -> p c b")

            # input loads go on the SP HWDGE ring (nc.sync); weight loads and
            # output stores on the ACT ring (nc.scalar) so they overlap.
            def load_block(i, split=False):
                # split=True (block 0): per-k-chunk DMAs so the first
                # matmuls can start as soon as their chunk lands
                b0 = i * NT
                xt_t = spool.tile([128, 8, NT], dt.bfloat16, tag="xt")
                nc.sync.dma_start(out=xt_t[:], in_=xt_r[:, :, b0 : b0 + NT])
                ot_t = spool.tile([128, 8, NT], dt.bfloat16, tag="ot")
                nc.sync.dma_start(out=ot_t[:], in_=ot_r[:, :, b0 : b0 + NT])
                lt_t = spool.tile([128, 3, NT], dt.bfloat16, tag="lt")
                nc.sync.dma_start(out=lt_t[:], in_=lt_r[:, :, b0 : b0 + NT])
                return xt_t, ot_t, lt_t

            # block-0 inputs + the weights the first matmuls need, first.
            # Weight DMA order mirrors consumption order; wenc is split in
            # half so the first enc matmuls can start sooner.
            # block-0 inputs + the big early weights share the sync ring,
            # interleaved in consumption order; everything else rides the
            # ACT ring concurrently.
            wenc_r = wenc_h[:].rearrange("k p m -> p k m")
            wenc_q = []
            # quarter 0 first, then x, then the rest just-in-time with the
            # other early weights: the sync ring delivers ~0.27 GB/us, so
            # order everything by first-consumption time
            q0 = cpool.tile([128, 8, 256], dt.bfloat16, tag="wenc_q0")
            nc.sync.dma_start(out=q0[:], in_=wenc_r[:, :, 0:256])
            wenc_q.append(q0)
            xt_t0 = spool.tile([128, 8, NT], dt.bfloat16, tag="xt")
            nc.sync.dma_start(out=xt_t0[:], in_=xt_r[:, :, 0:NT])
            for qi in range(1, 4):
                q = cpool.tile([128, 8, 256], dt.bfloat16, tag=f"wenc_q{qi}")
                nc.sync.dma_start(out=q[:], in_=wenc_r[:, :, qi * 256 : (qi + 1) * 256])
                wenc_q.append(q)
            ot_t0 = spool.tile([128, 8, NT], dt.bfloat16, tag="ot")
            nc.sync.dma_start(out=ot_t0[:], in_=ot_r[:, :, 0:NT])
            lt_t0 = spool.tile([128, 3, NT], dt.bfloat16, tag="lt")
            nc.sync.dma_start(out=lt_t0[:], in_=lt_r[:, :, 0:NT])
            wmulv_t = cpool.tile([128, 8, 2 * LAT], dt.bfloat16, tag="wmulv")
            nc.sync.dma_start(out=wmulv_t[:], in_=wmulv_h[:].rearrange("k p m -> p k m"))
            pending = (xt_t0, ot_t0, lt_t0)
            wopt_t = cpool.tile([128, 8, NA0], dt.bfloat16, tag="wopt")
            nc.scalar.dma_start(out=wopt_t[:], in_=wopt_h[:].rearrange("k p m -> p k m"))
            wlm_t = cpool.tile([128, 3, NA1], dt.bfloat16, tag="wlm")
            nc.scalar.dma_start(out=wlm_t[:], in_=wlm_h[:].rearrange("k p m -> p k m"))
            # tiny bias tensors next: epilogues block on them, and behind a
            # multi-MB weight they stall the whole pipeline (and re-throttle
            # the PE clock via HAM)
            ba_t = cpool.tile([128, 2], dt.float32, tag="ba")
            nc.scalar.dma_start(out=ba_t[:], in_=ba_h[:])
            benc_t = cpool.tile([128, 8], dt.float32, tag="benc")
            nc.scalar.dma_start(out=benc_t[:], in_=benc_h[:])
            bmulv_t = cpool.tile([128, 4], dt.float32, tag="bmulv")
            nc.scalar.dma_start(out=bmulv_t[:], in_=bmulv_h[:])
            bdec1_t = cpool.tile([128, 8], dt.float32, tag="bdec1")
            nc.scalar.dma_start(out=bdec1_t[:], in_=bdec1_h[:])
            bdec2_t = cpool.tile([128, 9], dt.float32, tag="bdec2")
            nc.scalar.dma_start(out=bdec2_t[:], in_=bdec2_h[:])
            wdec1_t = cpool.tile([128, 2, HID], dt.float8e4, tag="wdec1")
            nc.scalar.dma_start(out=wdec1_t[:], in_=wdec1_h[:].rearrange("k p m -> p k m"))
            s0_t = cpool.tile([NA0, OUT_DIM], dt.bfloat16, tag="s0")
            s1_t = cpool.tile([NA1, OUT_DIM], dt.bfloat16, tag="s1")
            wdec2_t = cpool.tile([128, 8, 1152], dt.float8e4, tag="wdec2")

            # PE warmup: dummy matmuls on the first-arrived weight tile keep
            # the PE busy while block-0 inputs stream in, so the HAM clock
            # gate reaches 8/8 before the real matmuls start. Results go to
            # a scratch PSUM tile that is never read.
            ps_warm = ppool.tile([128, NT], mybir.dt.float32, tag="ps")
            for w in range(16):
                nc.tensor.matmul(
                    ps_warm[:, 0:256],
                    wenc_q[0][:, 0, 0:128],
                    wenc_q[0][:, w % 8, 0:256],
                    start=(w == 0),
                    stop=(w == 15),
                )

            def compute_attn(ot_t, lt_t):
                # v-projections of the two seq-len-1 attention heads
                a0_t = spool.tile([NA0, NT], dt.bfloat16, tag="a0")
                ps = ppool.tile([NA0, NT], mybir.dt.float32, tag="ps")
                for k in range(8):
                    nc.tensor.matmul(
                        ps[:], wopt_t[:, k, :], ot_t[:, k, :],
                        start=(k == 0), stop=(k == 7),
                    )
                nc.vector.tensor_scalar_add(a0_t[:], ps[:], ba_t[:NA0, 0:1])
                a1_t = spool.tile([NA1, NT], dt.bfloat16, tag="a1")
                ps = ppool.tile([NA1, NT], mybir.dt.float32, tag="ps")
                for k in range(3):
                    nc.tensor.matmul(
                        ps[:], wlm_t[:, k, :], lt_t[:, k, :],
                        start=(k == 0), stop=(k == 2),
                    )
                nc.vector.tensor_scalar_add(a1_t[:], ps[:], ba_t[:NA1, 1:2])
                return a0_t, a1_t

            # which output chunks each scatter matrix actually touches
            s0_cols = set()
            for k, n in enumerate(_opt_nodes):
                for f in range(NF):
                    s0_cols.add(NODE * NODE + n * NF + f)
            for i, j in _OPT_EDGES:
                s0_cols.add(i * NODE + j)
            s1_cols = set()
            for k, n in enumerate(_log_nodes):
                for f in range(NF):
                    s1_cols.add(NODE * NODE + n * NF + f)
            for i, j in _LOG_EDGES:
                s1_cols.add(i * NODE + j)
            s0_nz = [any(m * 128 <= c < m * 128 + 128 for c in s0_cols) for m in range(9)]
            s1_nz = [any(m * 128 <= c < m * 128 + 128 for c in s1_cols) for m in range(9)]

            for i in range(NBLK):
                b0 = i * NT
                xt_t, ot_t, lt_t = pending

                # ---- encoder: hT = relu(enc_w.T @ xT + enc_b) ----
                ht_t = spool.tile([128, 8, NT], dt.bfloat16, tag="ht")
                for f in range(8):
                    ps = ppool.tile([128, NT], mybir.dt.float32, tag="ps")
                    for k in range(8):
                        wq = wenc_q[f // 2]
                        fo = f % 2
                        nc.tensor.matmul(
                            ps[:],
                            wq[:, k, fo * 128 : (fo + 1) * 128],
                            xt_t[:, k, :],
                            start=(k == 0),
                            stop=(k == 7),
                        )
                    nc.scalar.activation(
                        ht_t[:, f, :], ps[:], AF.Relu, bias=benc_t[:, f : f + 1]
                    )

                if i == 0:
                    # dec2 weights issued only now: keeps the startup window
                    # clear for the enc/attn-critical transfers
                    nc.scalar.dma_start(out=s0_t[:], in_=s0_h[:])
                    nc.scalar.dma_start(out=s1_t[:], in_=s1_h[:])
                    nc.scalar.dma_start(
                        out=wdec2_t[:], in_=wdec2_h[:].rearrange("k p m -> p k m")
                    )

                # attention heads here: independent PE work that covers the
                # enc->mulv epilogue dependency stall
                a0_t, a1_t = compute_attn(ot_t, lt_t)
                if i + 1 < NBLK:
                    pending = load_block(i + 1)

                # ---- mu | logvar (mu chunks also written as fp8 z) ----
                mulv_f = spool.tile([128, 4, NT], mybir.dt.float32, tag="mulvf")
                zt_t = spool.tile([128, 2, NT], dt.float8e4, tag="zt")
                for m in range(4):
                    ps = ppool.tile([128, NT], mybir.dt.float32, tag="ps")
                    for k in range(8):
                        nc.tensor.matmul(
                            ps[:],
                            wmulv_t[:, k, m * 128 : (m + 1) * 128],
                            ht_t[:, k, :],
                            start=(k == 0),
                            stop=(k == 7),
                        )
                    nc.vector.tensor_scalar_add(
                        mulv_f[:, m, :], ps[:], bmulv_t[:, m : m + 1]
                    )
                    if m < 2:
                        nc.vector.tensor_scalar_add(
                            zt_t[:, m, :], ps[:], bmulv_t[:, m : m + 1]
                        )
                nc.scalar.dma_start(out=mulvt_r[:, :, b0 : b0 + NT], in_=mulv_f[:])

                # ---- decoder layer 1 (fp8 DoubleRow) ----
                h2_t = spool.tile([128, 8, NT], dt.float8e4, tag="h2t")
                for f in range(8):
                    ps = ppool.tile([128, NT], mybir.dt.float32, tag="ps")
                    nc.tensor.matmul(
                        ps[:],
                        wdec1_t[:, 0:2, f * 128 : (f + 1) * 128],
                        zt_t[:, 0:2, :],
                        start=True,
                        stop=True,
                        perf_mode=mybir.MatmulPerfMode.DoubleRow,
                    )
                    nc.scalar.activation(
                        h2_t[:, f, :], ps[:], AF.Relu, bias=bdec1_t[:, f : f + 1]
                    )

                # ---- decoder layer 2 (fp8 DoubleRow) + scatter-add ----
                # scatter matmuls first: they only need a0/a1, so the PE can
                # run them while the last h2 epilogue is still finishing
                out_f = opool.tile([128, 9, NT], mybir.dt.float32, tag="outf")
                # scatter-led chunks first: their S matmuls depend only on
                # a0/a1 (ready early), covering the h2-epilogue latency that
                # otherwise stalls the first kc matmuls
                for m in [3, 4, 5, 6, 7, 8, 0, 1, 2]:
                    mw = 128 if m < 8 else OUT_DIM - 8 * 128  # 26
                    ps = ppool.tile([mw, NT], mybir.dt.float32, tag="ps")
                    first = True
                    if s0_nz[m]:
                        nc.tensor.matmul(
                            ps[:], s0_t[:, m * 128 : m * 128 + mw], a0_t[:],
                            start=first, stop=False,
                        )
                        first = False
                    if s1_nz[m]:
                        nc.tensor.matmul(
                            ps[:], s1_t[:, m * 128 : m * 128 + mw], a1_t[:],
                            start=first, stop=False,
                        )
                        first = False
                    for kc in range(4):
                        nc.tensor.matmul(
                            ps[:],
                            wdec2_t[:, 2 * kc : 2 * kc + 2, m * 128 : m * 128 + mw],
                            h2_t[:, 2 * kc : 2 * kc + 2, :],
                            start=first,
                            stop=(kc == 3),
                            perf_mode=mybir.MatmulPerfMode.DoubleRow,
                        )
                        first = False
                    nc.vector.tensor_scalar_add(
                        out_f[:mw, m, :], ps[:], bdec2_t[:mw, m : m + 1]
                    )
                    nc.scalar.dma_start(
                        out=outt_r[:mw, m, b0 : b0 + NT], in_=out_f[:mw, m, :]
                    )

    _legalize_waits(nc)
    return nc


def kernel(x, optical, log, params):
    global LAST_RESULTS
    from concourse.bass_utils import run_bass_kernel_spmd

    _install_ntff_shim()
    weights, xt, ot, lt = _prep_host(x, optical, log, params)
    nc = _build_graph()

    in_maps = []
    for c in range(N_CORES):
        sl = slice(c * BC, (c + 1) * BC)
        m = dict(weights)
        m["xt"] = np.ascontiguousarray(xt[:, :, sl])
        m["ot"] = np.ascontiguousarray(ot[:, :, sl])
        m["lt"] = np.ascontiguousarray(lt[:, :, sl])
        in_maps.append(m)

    try:
        res = run_bass_kernel_spmd(nc, in_maps, core_ids=list(range(N_CORES)))
    except Exception:
        # profiling-path hiccups (e.g. transient NTFF start/stop failures)
        # shouldn't take down the run — retry once with tracing disabled
        os.environ["BASS_NEVER_TRACE"] = "1"
        res = run_bass_kernel_spmd(nc, in_maps, core_ids=list(range(N_CORES)))
    LAST_RESULTS = res

    outt = np.concatenate([res.results[c]["outt"] for c in range(N_CORES)], axis=2)
    mulvt = np.concatenate([res.results[c]["mulvt"] for c in range(N_CORES)], axis=2)
    out = outt.reshape(9 * 128, B)[:OUT_DIM].T          # [B, 1050]
    mulv = mulvt.reshape(4 * 128, B)                     # [512, B]

    edge = np.ascontiguousarray(out[:, : NODE * NODE].reshape(B, NODE, NODE))
    node = np.ascontiguousarray(out[:, NODE * NODE :].reshape(B, NODE, NF))
    mu = np.ascontiguousarray(mulv[:LAT].T)
    logvar = np.ascontiguousarray(mulv[LAT : 2 * LAT].T)
    return edge, node, mu, logvar


# revision 46
# speedup vs baseline: 1.0457x; 1.0457x over previous
"""Trainium2 Bass kernel for nn_BAE (VAE-style encoder/decoder with fused
scatter-add attention heads), data-parallel over 8 NeuronCores.

Key algebraic simplifications applied on host:
- seq_len==1 attention: softmax over a singleton axis is exactly 1, so the
  attention output equals the v-projection; q/k matmuls are dropped.
- Eval-mode BatchNorm is an affine transform; it is folded into the next
  layer's weights/biases, so no BN ops run on device.
- The edge symmetrization (out + out^T)/2 is linear; it is folded into
  dec2's weight/bias.
- log.mean(axis=1) is folded into the log-attention v-weights (tiled /32).
- The fixed-index scatter-adds become one 0/1 scatter matrix S applied as
  extra PSUM-accumulating matmuls in the dec2 output group.

Device pipeline (per core, feature-major, bf16 matmul / fp32 PSUM):
  xT -> relu(enc) -> hT -> mu|lv -> zT -> relu(dec1) -> h2T -> dec2 + S@a
  aT = [optical@Wopt | logflat@Wlm] computed the same way.
"""

import os
import numpy as np

B = 32768
N_CORES = 8
BC = B // N_CORES          # 4096 samples per core
NT = 512                   # samples per block (matmul moving dim)
NBLK = BC // NT            # 8 blocks per core
D_IN = 1024
HID = 1024
LAT = 256
OUT_DIM = 1050
NODE = 30
NF = 5
BN_EPS = 1e-5

_opt_nodes = list(range(20, 30))
_OPT_EDGES = [(i, j) for i in _opt_nodes for j in _opt_nodes if i <= j]   # 55
_log_nodes = [20, 21, 22, 23, 24, 25, 26, 27, 28, 29, 14, 15]
_LOG_EDGES = [(i, j) for i in _log_nodes for j in _log_nodes if i <= j]   # 78

NA0 = 10 + len(_OPT_EDGES)     # 65  (optical head: 10 node + 55 edge)
NA1 = 12 + len(_LOG_EDGES)     # 90  (log head: 12 node + 78 edge)

LAST_RESULTS = None  # test harness reads exec_time_ns from here


def _install_ntff_shim():
    """Provide antenv.axon_hooks if the image lacks it, wiring the NTFF
    profile hook to the axon .so via the boot helper. Makes trace=True
    (BASS_TRACE=1) work instead of crashing on a missing import."""
    import sys
    import types

    try:
        from antenv.axon_hooks import get_axon_ntff_profile_hook  # noqa: F401

        return
    except ImportError:
        pass
    hook = None
    try:
        from trn_agent_boot.trn_boot import _ntff_profile_via_ctypes

        hook = _ntff_profile_via_ctypes("/opt/axon/libaxon_pjrt.so")
    except Exception:
        hook = None
    mod = types.ModuleType("antenv.axon_hooks")
    mod._hook = hook
    mod.get_axon_ntff_profile_hook = lambda: mod._hook
    def _set(h):
        mod._hook = h
    mod.set_axon_ntff_profile_hook = _set
    sys.modules["antenv.axon_hooks"] = mod
    try:
        import antenv

        antenv.axon_hooks = mod
    except ImportError:
        pass


def _make_tile_context_cls():
    """TileContext whose kernel-tail drain splits sem waits one-per-drain
    (this toolchain's walrus rejects >few waits on a CTRL instruction)."""
    import bass_rust
    from concourse.tile import TileContext
    from concourse.vector_clock import ScopedClock

    class TileContextSplitDrain(TileContext):
        def _drain_and_barrier(self, tick_clock, wait_clock):
            drain_inst = self.nc.sync.drain()
            wait_clock.add_sem_waits(
                drain_inst.ins, ScopedClock({None: tick_clock.global_clock})
            )
            si = drain_inst.ins.sync_info
            if si is not None:
                waits = list(si.on_wait or [])
                upds = list(si.on_update or [])
                if len(waits) > 1:
                    drain_inst.ins.sync_info = bass_rust.SyncInfo(
                        on_wait=waits[:1], on_update=upds
                    )
                    for i in range(1, len(waits)):
                        extra = self.nc.sync.drain()
                        extra.ins.sync_info = bass_rust.SyncInfo(
                            on_wait=waits[i : i + 1], on_update=[]
                        )
            self.nc.all_engine_barrier()
            assert self.sems is not None
            popped = self.nc._tile_sem_poison_stack.pop()
            assert popped is self._sem_poison
            self.nc.clear_and_free_semaphores(list(self.sems.allocated().values()))
            self.nc.all_engine_barrier()

    return TileContextSplitDrain


def _prep_host(x, optical, log, p):
    """Fold BN/symmetrization/mean, build scatter matrix, lay out weights
    and inputs in device-ready (feature-major, chunked) form."""
    import ml_dtypes

    bf16 = ml_dtypes.bfloat16
    f32 = np.float32

    def npa(a):
        return np.asarray(a, dtype=f32)

    x = npa(x)
    optical = npa(optical)
    log = npa(log)
    p = {k: npa(v) for k, v in p.items()}

    # ---- BN folds -------------------------------------------------------
    g1 = p["enc_bn_g"] / np.sqrt(p["enc_bn_v"] + BN_EPS)
    b1 = p["enc_bn_b"] - p["enc_bn_m"] * g1
    w_mu = g1[:, None] * p["mu_w"]
    b_mu = p["mu_b"] + b1 @ p["mu_w"]
    w_lv = g1[:, None] * p["lv_w"]
    b_lv = p["lv_b"] + b1 @ p["lv_w"]
    w_mulv = np.concatenate([w_mu, w_lv], axis=1)          # [1024, 512]
    b_mulv = np.concatenate([b_mu, b_lv])                   # [512]

    g2 = p["dec_bn_g"] / np.sqrt(p["dec_bn_v"] + BN_EPS)
    b2 = p["dec_bn_b"] - p["dec_bn_m"] * g2
    W = p["dec2_w"]                                         # [1024, 1050]
    E = W[:, : NODE * NODE].reshape(HID, NODE, NODE)
    Es = (E + E.transpose(0, 2, 1)) * 0.5
    w_sym = np.concatenate([Es.reshape(HID, NODE * NODE), W[:, NODE * NODE :]], axis=1)
    bE = p["dec2_b"][: NODE * NODE].reshape(NODE, NODE)
    bEs = (bE + bE.T) * 0.5
    b_sym = np.concatenate([bEs.reshape(-1), p["dec2_b"][NODE * NODE :]])
    w_dec2 = g2[:, None] * w_sym                            # [1024, 1050]
    b_dec2 = b_sym + b2 @ w_sym                             # [1050]

    # ---- attention v-heads (softmax==1 so out == v) ---------------------
    w_opt = np.concatenate([p["on_vw"], p["oe_vw"]], axis=1)   # [1024, 65]
    b_opt = np.concatenate([p["on_vb"], p["oe_vb"]])           # [65]
    w_cat = np.concatenate([p["ln_vw"], p["le_vw"]], axis=1)   # [9, 90]
    b_lm = np.concatenate([p["ln_vb"], p["le_vb"]])            # [90]
    # fold mean over T_LOG=32: logflat[b] @ w_lm_exp == mean(log) @ w_cat
    w_lm_exp = np.zeros((384, NA1), f32)
    w_lm_exp[:288] = np.tile(w_cat / 32.0, (32, 1))

    # ---- scatter matrices ----------------------------------------------
    S0 = np.zeros((NA0, OUT_DIM), f32)
    for k, n in enumerate(_opt_nodes):
        for f in range(NF):
            S0[k, NODE * NODE + n * NF + f] = 1.0
    for k, (i, j) in enumerate(_OPT_EDGES):
        S0[10 + k, i * NODE + j] = 1.0
    S1 = np.zeros((NA1, OUT_DIM), f32)
    for k, n in enumerate(_log_nodes):
        for f in range(NF):
            S1[k, NODE * NODE + n * NF + f] = 1.0
    for k, (i, j) in enumerate(_LOG_EDGES):
        S1[12 + k, i * NODE + j] = 1.0

    # ---- device layouts -------------------------------------------------
    fp8 = ml_dtypes.float8_e4m3

    def chunk_w(w, kchunks, m, dtype=bf16):
        # [K, M] -> [128, kchunks, M], partition-major so the weight DMA
        # reads one fully contiguous run per partition (fast descriptors)
        return np.ascontiguousarray(
            w.reshape(kchunks, 128, m).transpose(1, 0, 2).astype(dtype)
        )

    # dec2 padded to M=1152 so the DoubleRow pair-dim step is 16B-aligned
    w_dec2_pad = np.zeros((HID, 1152), f32)
    w_dec2_pad[:, :OUT_DIM] = w_dec2

    wenc_qm = np.ascontiguousarray(
        p["enc_w"].reshape(8, 128, 4, 256).transpose(2, 1, 0, 3).astype(bf16)
    )  # [quarter, 128, k, 256]
    weights = {
        "wenc": wenc_qm,
        "wmulv": chunk_w(w_mulv, 8, 2 * LAT),
        "wdec1": chunk_w(p["dec1_w"], 2, HID, fp8),
        "wdec2": chunk_w(w_dec2_pad, 8, 1152, fp8),
        "wopt": chunk_w(w_opt, 8, NA0),
        "wlm": chunk_w(w_lm_exp, 3, NA1),
        "s0": np.ascontiguousarray(S0.astype(bf16)),
        "s1": np.ascontiguousarray(S1.astype(bf16)),
        "benc": np.ascontiguousarray(p["enc_b"].reshape(8, 128).T),
        "bmulv": np.ascontiguousarray(b_mulv.reshape(4, 128).T),
        "bdec1": np.ascontiguousarray(p["dec1_b"].reshape(8, 128).T),
    }
    bdec2_pad = np.zeros(1152, f32)
    bdec2_pad[:OUT_DIM] = b_dec2
    weights["bdec2"] = np.ascontiguousarray(bdec2_pad.reshape(9, 128).T)
    ba = np.zeros((128, 2), f32)
    ba[:NA0, 0] = b_opt
    ba[:NA1, 1] = b_lm
    weights["ba"] = ba

    # inputs: block-major [nblk_total, 128, kchunks, NT] so each block load
    # is one DMA with a contiguous per-partition run
    nbt = B // NT
    xt = np.ascontiguousarray(
        x.reshape(nbt, NT, 8, 128).transpose(0, 3, 2, 1).astype(bf16)
    )
    ot = np.ascontiguousarray(
        optical.reshape(nbt, NT, 8, 128).transpose(0, 3, 2, 1).astype(bf16)
    )
    lf = np.zeros((B, 384), f32)
    lf[:, :288] = log.reshape(B, 288)
    lt = np.ascontiguousarray(
        lf.reshape(nbt, NT, 3, 128).transpose(0, 3, 2, 1).astype(bf16)
    )
    return weights, xt, ot, lt


def _legalize_waits(nc, max_waits=1):
    """This toolchain's walrus rejects instructions carrying more than a
    couple of sync waits. Hoist excess waits onto preceding same-engine
    NoOps (engines execute in order, so semantics are unchanged)."""
    import bass_rust
    import concourse.mybir as mybir

    ctr = 0
    for f in nc.m.functions:
        for bb in f.blocks:
            changed = False
            out = []
            for inst in bb.instructions:
                si = inst.sync_info
                waits = list(si.on_wait or []) if si is not None else []
                if len(waits) > max_waits:
                    changed = True
                    extra = waits[max_waits:]
                    for j in range(0, len(extra), max_waits):
                        ctr += 1
                        nop = mybir.InstNoOp(name=f"waitsplit-{ctr}", ins=[], outs=[])
                        nop.engine = inst.engine
                        nop.sync_info = bass_rust.SyncInfo(
                            on_wait=extra[j : j + max_waits], on_update=[]
                        )
                        out.append(nop)
                    inst.sync_info = bass_rust.SyncInfo(
                        on_wait=waits[:max_waits],
                        on_update=list(si.on_update or []),
                    )
                out.append(inst)
            if changed:
                bb.instructions = out


def _build_graph():
    import concourse.bass as bass
    import concourse.mybir as mybir

    TileCtx = _make_tile_context_cls()
    dt = mybir.dt
    AF = mybir.ActivationFunctionType

    nc = bass.Bass()
    # inputs (per-core shard shapes)
    xt_h = nc.declare_dram_parameter("xt", [NBLK, 128, 8, NT], dt.bfloat16, isOutput=False)
    ot_h = nc.declare_dram_parameter("ot", [NBLK, 128, 8, NT], dt.bfloat16, isOutput=False)
    lt_h = nc.declare_dram_parameter("lt", [NBLK, 128, 3, NT], dt.bfloat16, isOutput=False)
    wenc_h = nc.declare_dram_parameter("wenc", [4, 128, 8, 256], dt.bfloat16, isOutput=False)
    wmulv_h = nc.declare_dram_parameter("wmulv", [128, 8, 2 * LAT], dt.bfloat16, isOutput=False)
    wdec1_h = nc.declare_dram_parameter("wdec1", [128, 2, HID], dt.float8e4, isOutput=False)
    wdec2_h = nc.declare_dram_parameter("wdec2", [128, 8, 1152], dt.float8e4, isOutput=False)
    wopt_h = nc.declare_dram_parameter("wopt", [128, 8, NA0], dt.bfloat16, isOutput=False)
    wlm_h = nc.declare_dram_parameter("wlm", [128, 3, NA1], dt.bfloat16, isOutput=False)
    s0_h = nc.declare_dram_parameter("s0", [NA0, OUT_DIM], dt.bfloat16, isOutput=False)
    s1_h = nc.declare_dram_parameter("s1", [NA1, OUT_DIM], dt.bfloat16, isOutput=False)
    benc_h = nc.declare_dram_parameter("benc", [128, 8], dt.float32, isOutput=False)
    bmulv_h = nc.declare_dram_parameter("bmulv", [128, 4], dt.float32, isOutput=False)
    bdec1_h = nc.declare_dram_parameter("bdec1", [128, 8], dt.float32, isOutput=False)
    bdec2_h = nc.declare_dram_parameter("bdec2", [128, 9], dt.float32, isOutput=False)
    ba_h = nc.declare_dram_parameter("ba", [128, 2], dt.float32, isOutput=False)
    # outputs
    outt_h = nc.declare_dram_parameter("outt", [9, 128, BC], dt.float32, isOutput=True)
    mulvt_h = nc.declare_dram_parameter("mulvt", [4, 128, BC], dt.float32, isOutput=True)

    with TileCtx(nc) as tc:
        with (
            tc.tile_pool(name="const", bufs=1) as cpool,
            tc.tile_pool(name="stream", bufs=2) as spool,
            tc.tile_pool(name="outp", bufs=3) as opool,
            tc.tile_pool(name="psum", bufs=8, space="PSUM") as ppool,
        ):
            outt_r = outt_h[:].rearrange("c p b -> p c b")
            mulvt_r = mulvt_h[:].rearrange("c p b -> p c b")

            # input loads go on the SP HWDGE ring (nc.sync); weight loads and
            # output stores on the ACT ring (nc.scalar) so they overlap.
            def load_block(i):
                xt_t = spool.tile([128, 8, NT], dt.bfloat16, tag="xt")
                nc.sync.dma_start(out=xt_t[:], in_=xt_h[i])
                ot_t = spool.tile([128, 8, NT], dt.bfloat16, tag="ot")
                nc.sync.dma_start(out=ot_t[:], in_=ot_h[i])
                lt_t = spool.tile([128, 3, NT], dt.bfloat16, tag="lt")
                nc.sync.dma_start(out=lt_t[:], in_=lt_h[i])
                return xt_t, ot_t, lt_t

            # block-0 inputs + the weights the first matmuls need, first.
            # Weight DMA order mirrors consumption order; wenc is split in
            # half so the first enc matmuls can start sooner.
            # block-0 inputs + the big early weights share the sync ring,
            # interleaved in consumption order; everything else rides the
            # ACT ring concurrently.
            wenc_q = []
            q0 = cpool.tile([128, 8, 256], dt.bfloat16, tag="wenc_q0")
            nc.sync.dma_start(out=q0[:], in_=wenc_h[0])
            wenc_q.append(q0)
            xt_t0 = spool.tile([128, 8, NT], dt.bfloat16, tag="xt")
            nc.sync.dma_start(out=xt_t0[:], in_=xt_h[0])
            for qi in range(1, 4):
                q = cpool.tile([128, 8, 256], dt.bfloat16, tag=f"wenc_q{qi}")
                nc.sync.dma_start(out=q[:], in_=wenc_h[qi])
                wenc_q.append(q)
            ot_t0 = spool.tile([128, 8, NT], dt.bfloat16, tag="ot")
            nc.sync.dma_start(out=ot_t0[:], in_=ot_h[0])
            lt_t0 = spool.tile([128, 3, NT], dt.bfloat16, tag="lt")
            nc.sync.dma_start(out=lt_t0[:], in_=lt_h[0])
            wmulv_t = cpool.tile([128, 8, 2 * LAT], dt.bfloat16, tag="wmulv")
            nc.sync.dma_start(out=wmulv_t[:], in_=wmulv_h[:])
            pending = (xt_t0, ot_t0, lt_t0)
            wopt_t = cpool.tile([128, 8, NA0], dt.bfloat16, tag="wopt")
            nc.scalar.dma_start(out=wopt_t[:], in_=wopt_h[:])
            wlm_t = cpool.tile([128, 3, NA1], dt.bfloat16, tag="wlm")
            nc.scalar.dma_start(out=wlm_t[:], in_=wlm_h[:])
            # tiny bias tensors next: epilogues block on them, and behind a
            # multi-MB weight they stall the whole pipeline (and re-throttle
            # the PE clock via HAM)
            ba_t = cpool.tile([128, 2], dt.float32, tag="ba")
            nc.scalar.dma_start(out=ba_t[:], in_=ba_h[:])
            benc_t = cpool.tile([128, 8], dt.float32, tag="benc")
            nc.scalar.dma_start(out=benc_t[:], in_=benc_h[:])
            bmulv_t = cpool.tile([128, 4], dt.float32, tag="bmulv")
            nc.scalar.dma_start(out=bmulv_t[:], in_=bmulv_h[:])
            bdec1_t = cpool.tile([128, 8], dt.float32, tag="bdec1")
            nc.scalar.dma_start(out=bdec1_t[:], in_=bdec1_h[:])
            bdec2_t = cpool.tile([128, 9], dt.float32, tag="bdec2")
            nc.scalar.dma_start(out=bdec2_t[:], in_=bdec2_h[:])
            wdec1_t = cpool.tile([128, 2, HID], dt.float8e4, tag="wdec1")
            nc.scalar.dma_start(out=wdec1_t[:], in_=wdec1_h[:])
            s0_t = cpool.tile([NA0, OUT_DIM], dt.bfloat16, tag="s0")
            s1_t = cpool.tile([NA1, OUT_DIM], dt.bfloat16, tag="s1")
            wdec2_t = cpool.tile([128, 8, 1152], dt.float8e4, tag="wdec2")

            # PE warmup: dummy matmuls on the first-arrived weight tile keep
            # the PE busy while block-0 inputs stream in, so the HAM clock
            # gate reaches 8/8 before the real matmuls start. Results go to
            # a scratch PSUM tile that is never read.
            ps_warm = ppool.tile([128, NT], mybir.dt.float32, tag="ps")
            for w in range(16):
                nc.tensor.matmul(
                    ps_warm[:, 0:256],
                    wenc_q[0][:, 0, 0:128],
                    wenc_q[0][:, w % 8, 0:256],
                    start=(w == 0),
                    stop=(w == 15),
                )

            def compute_attn(ot_t, lt_t):
                # v-projections of the two seq-len-1 attention heads
                a0_t = spool.tile([NA0, NT], dt.bfloat16, tag="a0")
                ps = ppool.tile([NA0, NT], mybir.dt.float32, tag="ps")
                for k in range(8):
                    nc.tensor.matmul(
                        ps[:], wopt_t[:, k, :], ot_t[:, k, :],
                        start=(k == 0), stop=(k == 7),
                    )
                nc.vector.tensor_scalar_add(a0_t[:], ps[:], ba_t[:NA0, 0:1])
                a1_t = spool.tile([NA1, NT], dt.bfloat16, tag="a1")
                ps = ppool.tile([NA1, NT], mybir.dt.float32, tag="ps")
                for k in range(3):
                    nc.tensor.matmul(
                        ps[:], wlm_t[:, k, :], lt_t[:, k, :],
                        start=(k == 0), stop=(k == 2),
                    )
                nc.vector.tensor_scalar_add(a1_t[:], ps[:], ba_t[:NA1, 1:2])
                return a0_t, a1_t

            # which output chunks each scatter matrix actually touches
            s0_cols = set()
            for k, n in enumerate(_opt_nodes):
                for f in range(NF):
                    s0_cols.add(NODE * NODE + n * NF + f)
            for i, j in _OPT_EDGES:
                s0_cols.add(i * NODE + j)
            s1_cols = set()
            for k, n in enumerate(_log_nodes):
                for f in range(NF):
                    s1_cols.add(NODE * NODE + n * NF + f)
            for i, j in _LOG_EDGES:
                s1_cols.add(i * NODE + j)
            s0_nz = [any(m * 128 <= c < m * 128 + 128 for c in s0_cols) for m in range(9)]
            s1_nz = [any(m * 128 <= c < m * 128 + 128 for c in s1_cols) for m in range(9)]

            for i in range(NBLK):
                b0 = i * NT
                xt_t, ot_t, lt_t = pending

                # ---- encoder: hT = relu(enc_w.T @ xT + enc_b) ----
                ht_t = spool.tile([128, 8, NT], dt.bfloat16, tag="ht")
                for f in range(8):
                    ps = ppool.tile([128, NT], mybir.dt.float32, tag="ps")
                    for k in range(8):
                        wq = wenc_q[f // 2]
                        fo = f % 2
                        nc.tensor.matmul(
                            ps[:],
                            wq[:, k, fo * 128 : (fo + 1) * 128],
                            xt_t[:, k, :],
                            start=(k == 0),
                            stop=(k == 7),
                        )
                    nc.scalar.activation(
                        ht_t[:, f, :], ps[:], AF.Relu, bias=benc_t[:, f : f + 1]
                    )

                if i == 0:
                    # dec2 weights issued only now: keeps the startup window
                    # clear for the enc/attn-critical transfers
                    nc.scalar.dma_start(out=s0_t[:], in_=s0_h[:])
                    nc.scalar.dma_start(out=s1_t[:], in_=s1_h[:])
                    nc.scalar.dma_start(out=wdec2_t[:], in_=wdec2_h[:])

                # attention heads here: independent PE work that covers the
                # enc->mulv epilogue dependency stall
                a0_t, a1_t = compute_attn(ot_t, lt_t)
                if i + 1 < NBLK:
                    pending = load_block(i + 1)

                # ---- mu | logvar (mu chunks also written as fp8 z) ----
                mulv_f = spool.tile([128, 4, NT], mybir.dt.float32, tag="mulvf")
                zt_t = spool.tile([128, 2, NT], dt.float8e4, tag="zt")
                for m in range(4):
                    ps = ppool.tile([128, NT], mybir.dt.float32, tag="ps")
                    for k in range(8):
                        nc.tensor.matmul(
                            ps[:],
                            wmulv_t[:, k, m * 128 : (m + 1) * 128],
                            ht_t[:, k, :],
                            start=(k == 0),
                            stop=(k == 7),
                        )
                    nc.vector.tensor_scalar_add(
                        mulv_f[:, m, :], ps[:], bmulv_t[:, m : m + 1]
                    )
                    if m < 2:
                        nc.vector.tensor_scalar_add(
                            zt_t[:, m, :], ps[:], bmulv_t[:, m : m + 1]
                        )
                nc.scalar.dma_start(out=mulvt_r[:, :, b0 : b0 + NT], in_=mulv_f[:])

                # ---- decoder layer 1 (fp8 DoubleRow) ----
                h2_t = spool.tile([128, 8, NT], dt.float8e4, tag="h2t")
                for f in range(8):
                    ps = ppool.tile([128, NT], mybir.dt.float32, tag="ps")
                    nc.tensor.matmul(
                        ps[:],
                        wdec1_t[:, 0:2, f * 128 : (f + 1) * 128],
                        zt_t[:, 0:2, :],
                        start=True,
                        stop=True,
                        perf_mode=mybir.MatmulPerfMode.DoubleRow,
                    )
                    nc.scalar.activation(
                        h2_t[:, f, :], ps[:], AF.Relu, bias=bdec1_t[:, f : f + 1]
                    )

                # ---- decoder layer 2 (fp8 DoubleRow) + scatter-add ----
                # scatter matmuls first: they only need a0/a1, so the PE can
                # run them while the last h2 epilogue is still finishing
                out_f = opool.tile([128, 9, NT], mybir.dt.float32, tag="outf")
                # scatter-led chunks first: their S matmuls depend only on
                # a0/a1 (ready early), covering the h2-epilogue latency that
                # otherwise stalls the first kc matmuls
                for m in [3, 4, 5, 6, 7, 8, 0, 1, 2]:
                    mw = 128 if m < 8 else OUT_DIM - 8 * 128  # 26
                    ps = ppool.tile([mw, NT], mybir.dt.float32, tag="ps")
                    first = True
                    if s0_nz[m]:
                        nc.tensor.matmul(
                            ps[:], s0_t[:, m * 128 : m * 128 + mw], a0_t[:],
                            start=first, stop=False,
                        )
                        first = False
                    if s1_nz[m]:
                        nc.tensor.matmul(
                            ps[:], s1_t[:, m * 128 : m * 128 + mw], a1_t[:],
                            start=first, stop=False,
                        )
                        first = False
                    for kc in range(4):
                        nc.tensor.matmul(
                            ps[:],
                            wdec2_t[:, 2 * kc : 2 * kc + 2, m * 128 : m * 128 + mw],
                            h2_t[:, 2 * kc : 2 * kc + 2, :],
                            start=first,
                            stop=(kc == 3),
                            perf_mode=mybir.MatmulPerfMode.DoubleRow,
                        )
                        first = False
                    nc.vector.tensor_scalar_add(
                        out_f[:mw, m, :], ps[:], bdec2_t[:mw, m : m + 1]
                    )
                    nc.scalar.dma_start(
                        out=outt_r[:mw, m, b0 : b0 + NT], in_=out_f[:mw, m, :]
                    )

    _legalize_waits(nc)
    return nc


def kernel(x, optical, log, params):
    global LAST_RESULTS
    from concourse.bass_utils import run_bass_kernel_spmd

    _install_ntff_shim()
    weights, xt, ot, lt = _prep_host(x, optical, log, params)
    nc = _build_graph()

    in_maps = []
    for c in range(N_CORES):
        sl = slice(c * NBLK, (c + 1) * NBLK)
        m = dict(weights)
        m["xt"] = np.ascontiguousarray(xt[sl])
        m["ot"] = np.ascontiguousarray(ot[sl])
        m["lt"] = np.ascontiguousarray(lt[sl])
        in_maps.append(m)

    try:
        res = run_bass_kernel_spmd(nc, in_maps, core_ids=list(range(N_CORES)))
    except Exception:
        # profiling-path hiccups (e.g. transient NTFF start/stop failures)
        # shouldn't take down the run — retry once with tracing disabled
        os.environ["BASS_NEVER_TRACE"] = "1"
        res = run_bass_kernel_spmd(nc, in_maps, core_ids=list(range(N_CORES)))
    LAST_RESULTS = res

    outt = np.concatenate([res.results[c]["outt"] for c in range(N_CORES)], axis=2)
    mulvt = np.concatenate([res.results[c]["mulvt"] for c in range(N_CORES)], axis=2)
    out = outt.reshape(9 * 128, B)[:OUT_DIM].T          # [B, 1050]
    mulv = mulvt.reshape(4 * 128, B)                     # [512, B]

    edge = np.ascontiguousarray(out[:, : NODE * NODE].reshape(B, NODE, NODE))
    node = np.ascontiguousarray(out[:, NODE * NODE :].reshape(B, NODE, NF))
    mu = np.ascontiguousarray(mulv[:LAT].T)
    logvar = np.ascontiguousarray(mulv[LAT : 2 * LAT].T)
    return edge, node, mu, logvar


# revision 47
# speedup vs baseline: 1.0493x; 1.0034x over previous
"""Trainium2 Bass kernel for nn_BAE (VAE-style encoder/decoder with fused
scatter-add attention heads), data-parallel over 8 NeuronCores.

Key algebraic simplifications applied on host:
- seq_len==1 attention: softmax over a singleton axis is exactly 1, so the
  attention output equals the v-projection; q/k matmuls are dropped.
- Eval-mode BatchNorm is an affine transform; it is folded into the next
  layer's weights/biases, so no BN ops run on device.
- The edge symmetrization (out + out^T)/2 is linear; it is folded into
  dec2's weight/bias.
- log.mean(axis=1) is folded into the log-attention v-weights (tiled /32).
- The fixed-index scatter-adds become one 0/1 scatter matrix S applied as
  extra PSUM-accumulating matmuls in the dec2 output group.

Device pipeline (per core, feature-major, bf16 matmul / fp32 PSUM):
  xT -> relu(enc) -> hT -> mu|lv -> zT -> relu(dec1) -> h2T -> dec2 + S@a
  aT = [optical@Wopt | logflat@Wlm] computed the same way.
"""

import os
import numpy as np

B = 32768
N_CORES = 8
BC = B // N_CORES          # 4096 samples per core
NT = 512                   # samples per block (matmul moving dim)
NBLK = BC // NT            # 8 blocks per core
D_IN = 1024
HID = 1024
LAT = 256
OUT_DIM = 1050
NODE = 30
NF = 5
BN_EPS = 1e-5

_opt_nodes = list(range(20, 30))
_OPT_EDGES = [(i, j) for i in _opt_nodes for j in _opt_nodes if i <= j]   # 55
_log_nodes = [20, 21, 22, 23, 24, 25, 26, 27, 28, 29, 14, 15]
_LOG_EDGES = [(i, j) for i in _log_nodes for j in _log_nodes if i <= j]   # 78

NA0 = 10 + len(_OPT_EDGES)     # 65  (optical head: 10 node + 55 edge)
NA1 = 12 + len(_LOG_EDGES)     # 90  (log head: 12 node + 78 edge)

LAST_RESULTS = None  # test harness reads exec_time_ns from here


def _install_ntff_shim():
    """Provide antenv.axon_hooks if the image lacks it, wiring the NTFF
    profile hook to the axon .so via the boot helper. Makes trace=True
    (BASS_TRACE=1) work instead of crashing on a missing import."""
    import sys
    import types

    try:
        from antenv.axon_hooks import get_axon_ntff_profile_hook  # noqa: F401

        return
    except ImportError:
        pass
    hook = None
    try:
        from trn_agent_boot.trn_boot import _ntff_profile_via_ctypes

        hook = _ntff_profile_via_ctypes("/opt/axon/libaxon_pjrt.so")
    except Exception:
        hook = None
    mod = types.ModuleType("antenv.axon_hooks")
    mod._hook = hook
    mod.get_axon_ntff_profile_hook = lambda: mod._hook
    def _set(h):
        mod._hook = h
    mod.set_axon_ntff_profile_hook = _set
    sys.modules["antenv.axon_hooks"] = mod
    try:
        import antenv

        antenv.axon_hooks = mod
    except ImportError:
        pass


def _make_tile_context_cls():
    """TileContext whose kernel-tail drain splits sem waits one-per-drain
    (this toolchain's walrus rejects >few waits on a CTRL instruction)."""
    import bass_rust
    from concourse.tile import TileContext
    from concourse.vector_clock import ScopedClock

    class TileContextSplitDrain(TileContext):
        def _drain_and_barrier(self, tick_clock, wait_clock):
            drain_inst = self.nc.sync.drain()
            wait_clock.add_sem_waits(
                drain_inst.ins, ScopedClock({None: tick_clock.global_clock})
            )
            si = drain_inst.ins.sync_info
            if si is not None:
                waits = list(si.on_wait or [])
                upds = list(si.on_update or [])
                if len(waits) > 1:
                    drain_inst.ins.sync_info = bass_rust.SyncInfo(
                        on_wait=waits[:1], on_update=upds
                    )
                    for i in range(1, len(waits)):
                        extra = self.nc.sync.drain()
                        extra.ins.sync_info = bass_rust.SyncInfo(
                            on_wait=waits[i : i + 1], on_update=[]
                        )
            self.nc.all_engine_barrier()
            assert self.sems is not None
            popped = self.nc._tile_sem_poison_stack.pop()
            assert popped is self._sem_poison
            self.nc.clear_and_free_semaphores(list(self.sems.allocated().values()))
            self.nc.all_engine_barrier()

    return TileContextSplitDrain


def _prep_host(x, optical, log, p):
    """Fold BN/symmetrization/mean, build scatter matrix, lay out weights
    and inputs in device-ready (feature-major, chunked) form."""
    import ml_dtypes

    bf16 = ml_dtypes.bfloat16
    f32 = np.float32

    def npa(a):
        return np.asarray(a, dtype=f32)

    x = npa(x)
    optical = npa(optical)
    log = npa(log)
    p = {k: npa(v) for k, v in p.items()}

    # ---- BN folds -------------------------------------------------------
    g1 = p["enc_bn_g"] / np.sqrt(p["enc_bn_v"] + BN_EPS)
    b1 = p["enc_bn_b"] - p["enc_bn_m"] * g1
    w_mu = g1[:, None] * p["mu_w"]
    b_mu = p["mu_b"] + b1 @ p["mu_w"]
    w_lv = g1[:, None] * p["lv_w"]
    b_lv = p["lv_b"] + b1 @ p["lv_w"]
    w_mulv = np.concatenate([w_mu, w_lv], axis=1)          # [1024, 512]
    b_mulv = np.concatenate([b_mu, b_lv])                   # [512]

    g2 = p["dec_bn_g"] / np.sqrt(p["dec_bn_v"] + BN_EPS)
    b2 = p["dec_bn_b"] - p["dec_bn_m"] * g2
    W = p["dec2_w"]                                         # [1024, 1050]
    E = W[:, : NODE * NODE].reshape(HID, NODE, NODE)
    Es = (E + E.transpose(0, 2, 1)) * 0.5
    w_sym = np.concatenate([Es.reshape(HID, NODE * NODE), W[:, NODE * NODE :]], axis=1)
    bE = p["dec2_b"][: NODE * NODE].reshape(NODE, NODE)
    bEs = (bE + bE.T) * 0.5
    b_sym = np.concatenate([bEs.reshape(-1), p["dec2_b"][NODE * NODE :]])
    w_dec2 = g2[:, None] * w_sym                            # [1024, 1050]
    b_dec2 = b_sym + b2 @ w_sym                             # [1050]

    # ---- attention v-heads (softmax==1 so out == v) ---------------------
    w_opt = np.concatenate([p["on_vw"], p["oe_vw"]], axis=1)   # [1024, 65]
    b_opt = np.concatenate([p["on_vb"], p["oe_vb"]])           # [65]
    w_cat = np.concatenate([p["ln_vw"], p["le_vw"]], axis=1)   # [9, 90]
    b_lm = np.concatenate([p["ln_vb"], p["le_vb"]])            # [90]
    # fold mean over T_LOG=32: logflat[b] @ w_lm_exp == mean(log) @ w_cat
    w_lm_exp = np.zeros((384, NA1), f32)
    w_lm_exp[:288] = np.tile(w_cat / 32.0, (32, 1))

    # ---- scatter matrices ----------------------------------------------
    S0 = np.zeros((NA0, OUT_DIM), f32)
    for k, n in enumerate(_opt_nodes):
        for f in range(NF):
            S0[k, NODE * NODE + n * NF + f] = 1.0
    for k, (i, j) in enumerate(_OPT_EDGES):
        S0[10 + k, i * NODE + j] = 1.0
    S1 = np.zeros((NA1, OUT_DIM), f32)
    for k, n in enumerate(_log_nodes):
        for f in range(NF):
            S1[k, NODE * NODE + n * NF + f] = 1.0
    for k, (i, j) in enumerate(_LOG_EDGES):
        S1[12 + k, i * NODE + j] = 1.0

    # ---- device layouts -------------------------------------------------
    fp8 = ml_dtypes.float8_e4m3

    def chunk_w(w, kchunks, m, dtype=bf16):
        # [K, M] -> [128, kchunks, M], partition-major so the weight DMA
        # reads one fully contiguous run per partition (fast descriptors)
        return np.ascontiguousarray(
            w.reshape(kchunks, 128, m).transpose(1, 0, 2).astype(dtype)
        )

    # dec2 padded to M=1152 so the DoubleRow pair-dim step is 16B-aligned
    w_dec2_pad = np.zeros((HID, 1152), f32)
    w_dec2_pad[:, :OUT_DIM] = w_dec2

    wenc_qm = np.ascontiguousarray(
        p["enc_w"].reshape(8, 128, 4, 256).transpose(2, 1, 0, 3).astype(bf16)
    )  # [quarter, 128, k, 256]
    weights = {
        "wenc": wenc_qm,
        "wmulv": chunk_w(w_mulv, 8, 2 * LAT),
        "wdec1": chunk_w(p["dec1_w"], 2, HID, fp8),
        "wdec2": chunk_w(w_dec2_pad, 8, 1152, fp8),
        "wopt": chunk_w(w_opt, 8, NA0),
        "wlm": chunk_w(w_lm_exp, 3, NA1),
        "s0": np.ascontiguousarray(S0.astype(bf16)),
        "s1": np.ascontiguousarray(S1.astype(bf16)),
        "benc": np.ascontiguousarray(p["enc_b"].reshape(8, 128).T),
        "bmulv": np.ascontiguousarray(b_mulv.reshape(4, 128).T),
        "bdec1": np.ascontiguousarray(p["dec1_b"].reshape(8, 128).T),
    }
    bdec2_pad = np.zeros(1152, f32)
    bdec2_pad[:OUT_DIM] = b_dec2
    weights["bdec2"] = np.ascontiguousarray(bdec2_pad.reshape(9, 128).T)
    ba = np.zeros((128, 2), f32)
    ba[:NA0, 0] = b_opt
    ba[:NA1, 1] = b_lm
    weights["ba"] = ba

    # inputs: block-major [nblk_total, 128, kchunks, NT] so each block load
    # is one DMA with a contiguous per-partition run
    nbt = B // NT
    xt = np.ascontiguousarray(
        x.reshape(nbt, NT, 8, 128).transpose(0, 3, 2, 1).astype(bf16)
    )
    ot = np.ascontiguousarray(
        optical.reshape(nbt, NT, 8, 128).transpose(0, 3, 2, 1).astype(bf16)
    )
    lf = np.zeros((B, 384), f32)
    lf[:, :288] = log.reshape(B, 288)
    lt = np.ascontiguousarray(
        lf.reshape(nbt, NT, 3, 128).transpose(0, 3, 2, 1).astype(bf16)
    )
    return weights, xt, ot, lt


def _legalize_waits(nc, max_waits=1):
    """This toolchain's walrus rejects instructions carrying more than a
    couple of sync waits. Hoist excess waits onto preceding same-engine
    NoOps (engines execute in order, so semantics are unchanged)."""
    import bass_rust
    import concourse.mybir as mybir

    ctr = 0
    for f in nc.m.functions:
        for bb in f.blocks:
            changed = False
            out = []
            for inst in bb.instructions:
                si = inst.sync_info
                waits = list(si.on_wait or []) if si is not None else []
                if len(waits) > max_waits:
                    changed = True
                    extra = waits[max_waits:]
                    for j in range(0, len(extra), max_waits):
                        ctr += 1
                        nop = mybir.InstNoOp(name=f"waitsplit-{ctr}", ins=[], outs=[])
                        nop.engine = inst.engine
                        nop.sync_info = bass_rust.SyncInfo(
                            on_wait=extra[j : j + max_waits], on_update=[]
                        )
                        out.append(nop)
                    inst.sync_info = bass_rust.SyncInfo(
                        on_wait=waits[:max_waits],
                        on_update=list(si.on_update or []),
                    )
                out.append(inst)
            if changed:
                bb.instructions = out


def _build_graph():
    import concourse.bass as bass
    import concourse.mybir as mybir

    TileCtx = _make_tile_context_cls()
    dt = mybir.dt
    AF = mybir.ActivationFunctionType

    nc = bass.Bass()
    # inputs (per-core shard shapes)
    xt_h = nc.declare_dram_parameter("xt", [NBLK, 128, 8, NT], dt.bfloat16, isOutput=False)
    ot_h = nc.declare_dram_parameter("ot", [NBLK, 128, 8, NT], dt.bfloat16, isOutput=False)
    lt_h = nc.declare_dram_parameter("lt", [NBLK, 128, 3, NT], dt.bfloat16, isOutput=False)
    wenc_h = nc.declare_dram_parameter("wenc", [4, 128, 8, 256], dt.bfloat16, isOutput=False)
    wmulv_h = nc.declare_dram_parameter("wmulv", [128, 8, 2 * LAT], dt.bfloat16, isOutput=False)
    wdec1_h = nc.declare_dram_parameter("wdec1", [128, 2, HID], dt.float8e4, isOutput=False)
    wdec2_h = nc.declare_dram_parameter("wdec2", [128, 8, 1152], dt.float8e4, isOutput=False)
    wopt_h = nc.declare_dram_parameter("wopt", [128, 8, NA0], dt.bfloat16, isOutput=False)
    wlm_h = nc.declare_dram_parameter("wlm", [128, 3, NA1], dt.bfloat16, isOutput=False)
    s0_h = nc.declare_dram_parameter("s0", [NA0, OUT_DIM], dt.bfloat16, isOutput=False)
    s1_h = nc.declare_dram_parameter("s1", [NA1, OUT_DIM], dt.bfloat16, isOutput=False)
    benc_h = nc.declare_dram_parameter("benc", [128, 8], dt.float32, isOutput=False)
    bmulv_h = nc.declare_dram_parameter("bmulv", [128, 4], dt.float32, isOutput=False)
    bdec1_h = nc.declare_dram_parameter("bdec1", [128, 8], dt.float32, isOutput=False)
    bdec2_h = nc.declare_dram_parameter("bdec2", [128, 9], dt.float32, isOutput=False)
    ba_h = nc.declare_dram_parameter("ba", [128, 2], dt.float32, isOutput=False)
    # outputs
    outt_h = nc.declare_dram_parameter("outt", [9, 128, BC], dt.float32, isOutput=True)
    mulvt_h = nc.declare_dram_parameter("mulvt", [4, 128, BC], dt.float32, isOutput=True)

    with TileCtx(nc) as tc:
        with (
            tc.tile_pool(name="const", bufs=1) as cpool,
            tc.tile_pool(name="stream", bufs=2) as spool,
            tc.tile_pool(name="outp", bufs=3) as opool,
            tc.tile_pool(name="psum", bufs=8, space="PSUM") as ppool,
        ):
            outt_r = outt_h[:].rearrange("c p b -> p c b")
            mulvt_r = mulvt_h[:].rearrange("c p b -> p c b")

            # input loads go on the SP HWDGE ring (nc.sync); weight loads and
            # output stores on the ACT ring (nc.scalar) so they overlap.
            def load_block(i):
                xt_t = spool.tile([128, 8, NT], dt.bfloat16, tag="xt")
                nc.sync.dma_start(out=xt_t[:], in_=xt_h[i])
                ot_t = spool.tile([128, 8, NT], dt.bfloat16, tag="ot")
                nc.sync.dma_start(out=ot_t[:], in_=ot_h[i])
                lt_t = spool.tile([128, 3, NT], dt.bfloat16, tag="lt")
                nc.sync.dma_start(out=lt_t[:], in_=lt_h[i])
                return xt_t, ot_t, lt_t

            # block-0 inputs + the weights the first matmuls need, first.
            # Weight DMA order mirrors consumption order; wenc is split in
            # half so the first enc matmuls can start sooner.
            # block-0 inputs + the big early weights share the sync ring,
            # interleaved in consumption order; everything else rides the
            # ACT ring concurrently.
            wenc_q = []
            q0 = cpool.tile([128, 8, 256], dt.bfloat16, tag="wenc_q0")
            nc.sync.dma_start(out=q0[:], in_=wenc_h[0])
            wenc_q.append(q0)
            xt_t0 = spool.tile([128, 8, NT], dt.bfloat16, tag="xt")
            nc.sync.dma_start(out=xt_t0[:], in_=xt_h[0])
            for qi in range(1, 4):
                q = cpool.tile([128, 8, 256], dt.bfloat16, tag=f"wenc_q{qi}")
                nc.sync.dma_start(out=q[:], in_=wenc_h[qi])
                wenc_q.append(q)
            ot_t0 = spool.tile([128, 8, NT], dt.bfloat16, tag="ot")
            nc.sync.dma_start(out=ot_t0[:], in_=ot_h[0])
            lt_t0 = spool.tile([128, 3, NT], dt.bfloat16, tag="lt")
            nc.sync.dma_start(out=lt_t0[:], in_=lt_h[0])
            wmulv_t = cpool.tile([128, 8, 2 * LAT], dt.bfloat16, tag="wmulv")
            nc.sync.dma_start(out=wmulv_t[:], in_=wmulv_h[:])
            pending = (xt_t0, ot_t0, lt_t0)
            wopt_t = cpool.tile([128, 8, NA0], dt.bfloat16, tag="wopt")
            nc.scalar.dma_start(out=wopt_t[:], in_=wopt_h[:])
            wlm_t = cpool.tile([128, 3, NA1], dt.bfloat16, tag="wlm")
            nc.scalar.dma_start(out=wlm_t[:], in_=wlm_h[:])
            # tiny bias tensors next: epilogues block on them, and behind a
            # multi-MB weight they stall the whole pipeline (and re-throttle
            # the PE clock via HAM)
            ba_t = cpool.tile([128, 2], dt.float32, tag="ba")
            nc.scalar.dma_start(out=ba_t[:], in_=ba_h[:])
            benc_t = cpool.tile([128, 8], dt.float32, tag="benc")
            nc.scalar.dma_start(out=benc_t[:], in_=benc_h[:])
            bmulv_t = cpool.tile([128, 4], dt.float32, tag="bmulv")
            nc.scalar.dma_start(out=bmulv_t[:], in_=bmulv_h[:])
            bdec1_t = cpool.tile([128, 8], dt.float32, tag="bdec1")
            nc.scalar.dma_start(out=bdec1_t[:], in_=bdec1_h[:])
            bdec2_t = cpool.tile([128, 9], dt.float32, tag="bdec2")
            nc.scalar.dma_start(out=bdec2_t[:], in_=bdec2_h[:])
            wdec1_t = cpool.tile([128, 2, HID], dt.float8e4, tag="wdec1")
            nc.scalar.dma_start(out=wdec1_t[:], in_=wdec1_h[:])
            s0_t = cpool.tile([NA0, OUT_DIM], dt.bfloat16, tag="s0")
            s1_t = cpool.tile([NA1, OUT_DIM], dt.bfloat16, tag="s1")
            wdec2_t = cpool.tile([128, 8, 1152], dt.float8e4, tag="wdec2")

            # PE warmup: dummy matmuls on the first-arrived weight tile keep
            # the PE busy while block-0 inputs stream in, so the HAM clock
            # gate reaches 8/8 before the real matmuls start. Results go to
            # a scratch PSUM tile that is never read.
            ps_warm = ppool.tile([128, NT], mybir.dt.float32, tag="ps")
            for w in range(16):
                nc.tensor.matmul(
                    ps_warm[:, 0:256],
                    wenc_q[0][:, 0, 0:128],
                    wenc_q[0][:, w % 8, 0:256],
                    start=(w == 0),
                    stop=(w == 15),
                )

            def compute_attn(ot_t, lt_t):
                # v-projections of the two seq-len-1 attention heads
                a0_t = spool.tile([NA0, NT], dt.bfloat16, tag="a0")
                ps = ppool.tile([NA0, NT], mybir.dt.float32, tag="ps")
                for k in range(8):
                    nc.tensor.matmul(
                        ps[:], wopt_t[:, k, :], ot_t[:, k, :],
                        start=(k == 0), stop=(k == 7),
                    )
                nc.vector.tensor_scalar_add(a0_t[:], ps[:], ba_t[:NA0, 0:1])
                a1_t = spool.tile([NA1, NT], dt.bfloat16, tag="a1")
                ps = ppool.tile([NA1, NT], mybir.dt.float32, tag="ps")
                for k in range(3):
                    nc.tensor.matmul(
                        ps[:], wlm_t[:, k, :], lt_t[:, k, :],
                        start=(k == 0), stop=(k == 2),
                    )
                nc.vector.tensor_scalar_add(a1_t[:], ps[:], ba_t[:NA1, 1:2])
                return a0_t, a1_t

            # which output chunks each scatter matrix actually touches
            s0_cols = set()
            for k, n in enumerate(_opt_nodes):
                for f in range(NF):
                    s0_cols.add(NODE * NODE + n * NF + f)
            for i, j in _OPT_EDGES:
                s0_cols.add(i * NODE + j)
            s1_cols = set()
            for k, n in enumerate(_log_nodes):
                for f in range(NF):
                    s1_cols.add(NODE * NODE + n * NF + f)
            for i, j in _LOG_EDGES:
                s1_cols.add(i * NODE + j)
            s0_nz = [any(m * 128 <= c < m * 128 + 128 for c in s0_cols) for m in range(9)]
            s1_nz = [any(m * 128 <= c < m * 128 + 128 for c in s1_cols) for m in range(9)]

            for i in range(NBLK):
                b0 = i * NT
                xt_t, ot_t, lt_t = pending

                # ---- encoder: hT = relu(enc_w.T @ xT + enc_b) ----
                ht_t = spool.tile([128, 8, NT], dt.bfloat16, tag="ht")
                for f in range(8):
                    ps = ppool.tile([128, NT], mybir.dt.float32, tag="ps")
                    for k in range(8):
                        wq = wenc_q[f // 2]
                        fo = f % 2
                        nc.tensor.matmul(
                            ps[:],
                            wq[:, k, fo * 128 : (fo + 1) * 128],
                            xt_t[:, k, :],
                            start=(k == 0),
                            stop=(k == 7),
                        )
                    nc.scalar.activation(
                        ht_t[:, f, :], ps[:], AF.Relu, bias=benc_t[:, f : f + 1]
                    )

                if i == 0:
                    # dec2 weights issued only now: keeps the startup window
                    # clear for the enc/attn-critical transfers
                    nc.scalar.dma_start(out=s0_t[:], in_=s0_h[:])
                    nc.scalar.dma_start(out=s1_t[:], in_=s1_h[:])
                    nc.scalar.dma_start(out=wdec2_t[:], in_=wdec2_h[:])

                # attention heads here: independent PE work that covers the
                # enc->mulv epilogue dependency stall
                a0_t, a1_t = compute_attn(ot_t, lt_t)
                if i + 1 < NBLK:
                    pending = load_block(i + 1)

                # ---- mu | logvar (mu chunks also written as fp8 z) ----
                mulv_f = spool.tile([128, 4, NT], mybir.dt.float32, tag="mulvf")
                zt_t = spool.tile([128, 2, NT], dt.float8e4, tag="zt")
                for m in range(4):
                    ps = ppool.tile([128, NT], mybir.dt.float32, tag="ps")
                    for k in range(8):
                        nc.tensor.matmul(
                            ps[:],
                            wmulv_t[:, k, m * 128 : (m + 1) * 128],
                            ht_t[:, k, :],
                            start=(k == 0),
                            stop=(k == 7),
                        )
                    nc.vector.tensor_scalar_add(
                        mulv_f[:, m, :], ps[:], bmulv_t[:, m : m + 1]
                    )
                    if m < 2:
                        nc.vector.tensor_scalar_add(
                            zt_t[:, m, :], ps[:], bmulv_t[:, m : m + 1]
                        )
                nc.scalar.dma_start(out=mulvt_r[:, :, b0 : b0 + NT], in_=mulv_f[:])

                # ---- decoder layer 1 (fp8 DoubleRow) ----
                h2_t = spool.tile([128, 8, NT], dt.float8e4, tag="h2t")
                for f in range(8):
                    ps = ppool.tile([128, NT], mybir.dt.float32, tag="ps")
                    nc.tensor.matmul(
                        ps[:],
                        wdec1_t[:, 0:2, f * 128 : (f + 1) * 128],
                        zt_t[:, 0:2, :],
                        start=True,
                        stop=True,
                        perf_mode=mybir.MatmulPerfMode.DoubleRow,
                    )
                    nc.scalar.activation(
                        h2_t[:, f, :], ps[:], AF.Relu, bias=bdec1_t[:, f : f + 1]
                    )

                # ---- decoder layer 2 (fp8 DoubleRow) + scatter-add ----
                # scatter matmuls first: they only need a0/a1, so the PE can
                # run them while the last h2 epilogue is still finishing
                out_f = opool.tile([128, 9, NT], mybir.dt.float32, tag="outf")
                # scatter-led chunks first: their S matmuls depend only on
                # a0/a1 (ready early), covering the h2-epilogue latency that
                # otherwise stalls the first kc matmuls
                for m in [3, 4, 5, 6, 7, 0, 1, 2, 8]:
                    mw = 128 if m < 8 else OUT_DIM - 8 * 128  # 26
                    ps = ppool.tile([mw, NT], mybir.dt.float32, tag="ps")
                    first = True
                    if s0_nz[m]:
                        nc.tensor.matmul(
                            ps[:], s0_t[:, m * 128 : m * 128 + mw], a0_t[:],
                            start=first, stop=False,
                        )
                        first = False
                    if s1_nz[m]:
                        nc.tensor.matmul(
                            ps[:], s1_t[:, m * 128 : m * 128 + mw], a1_t[:],
                            start=first, stop=False,
                        )
                        first = False
                    for kc in range(4):
                        nc.tensor.matmul(
                            ps[:],
                            wdec2_t[:, 2 * kc : 2 * kc + 2, m * 128 : m * 128 + mw],
                            h2_t[:, 2 * kc : 2 * kc + 2, :],
                            start=first,
                            stop=(kc == 3),
                            perf_mode=mybir.MatmulPerfMode.DoubleRow,
                        )
                        first = False
                    nc.vector.tensor_scalar_add(
                        out_f[:mw, m, :], ps[:], bdec2_t[:mw, m : m + 1]
                    )
                    nc.scalar.dma_start(
                        out=outt_r[:mw, m, b0 : b0 + NT], in_=out_f[:mw, m, :]
                    )

    _legalize_waits(nc)
    return nc


def kernel(x, optical, log, params):
    global LAST_RESULTS
    from concourse.bass_utils import run_bass_kernel_spmd

    _install_ntff_shim()
    weights, xt, ot, lt = _prep_host(x, optical, log, params)
    nc = _build_graph()

    in_maps = []
    for c in range(N_CORES):
        sl = slice(c * NBLK, (c + 1) * NBLK)
        m = dict(weights)
        m["xt"] = np.ascontiguousarray(xt[sl])
        m["ot"] = np.ascontiguousarray(ot[sl])
        m["lt"] = np.ascontiguousarray(lt[sl])
        in_maps.append(m)

    try:
        res = run_bass_kernel_spmd(nc, in_maps, core_ids=list(range(N_CORES)))
    except Exception:
        # profiling-path hiccups (e.g. transient NTFF start/stop failures)
        # shouldn't take down the run — retry once with tracing disabled
        os.environ["BASS_NEVER_TRACE"] = "1"
        res = run_bass_kernel_spmd(nc, in_maps, core_ids=list(range(N_CORES)))
    LAST_RESULTS = res

    outt = np.concatenate([res.results[c]["outt"] for c in range(N_CORES)], axis=2)
    mulvt = np.concatenate([res.results[c]["mulvt"] for c in range(N_CORES)], axis=2)
    out = outt.reshape(9 * 128, B)[:OUT_DIM].T          # [B, 1050]
    mulv = mulvt.reshape(4 * 128, B)                     # [512, B]

    edge = np.ascontiguousarray(out[:, : NODE * NODE].reshape(B, NODE, NODE))
    node = np.ascontiguousarray(out[:, NODE * NODE :].reshape(B, NODE, NF))
    mu = np.ascontiguousarray(mulv[:LAT].T)
    logvar = np.ascontiguousarray(mulv[LAT : 2 * LAT].T)
    return edge, node, mu, logvar
